# revision 33
# baseline (speedup 1.0000x reference)
"""Trainium2 Bass kernel for nn_Evo_Path_GNN (gnn_message_passing).

Algorithm
---------
The reference runs a 50000-step sequential scan over edges on a [10, 256]
state.  Each step is affine in the state row it touches:

    state[n] <- (state[n] + b) @ U        (one "touch"; 2 touches per edge)

with b = inv_deg[n] * msg[e] * node_feat[partner].  Unrolling per node, the
final row is

    out[n] = node_feat[n] @ U^{m_n} + sum_k b_{n,k} @ U^{m_n - k + 1}

where m_n is the number of touches of node n and k the touch order.  U is
0.01-scaled gaussian (||U^k|| decays ~5x per power), so terms older than a
few touches are below the accuracy gate.  We keep only the last K touches
per node (K chosen at runtime from the measured norms of U^k), which
converts the 100k-long serial chain into the closed form

    out[n] = sum_{j'=0}^{K-1} P_{n,j'} @ U^{j'+1} + base_n

with P_{n,j'} the b-vector of the (m_n - j')-th touch of node n.  Unlike the
earlier Horner-recursion kernel, this evaluates the sum DIRECTLY as a flat
accumulation of K*2 independent PSUM-accumulating matmuls

    out[10,256] += bt[:, a, j-slice].T @ Upow[j][half a]    (a=0,1; j=0..K-1)

with the tiny [128,10] bt slices as the PE *stationary* operand and the
U-powers as 256-wide *moving* operands.  This removes the serial PE
dependency chain, produces the output in natural [10, 256] layout (10 fat
DMA descriptors instead of 256 40-byte ones), and needs no Horner adds.
U powers and Wc = messageNN @ intsc_feat_fc^T are folded on the host
(weight-only O(D^3) preprocessing); the host also does all integer
bookkeeping (touch ordering, slot permutation, gathers) and the low-degree
boundary constants (node_feat @ U^m for nodes with m <= K touches).  The
device does all per-edge feature math of the truncated algorithm: the
message projection matmuls, the b-vector products, and the K-power update
algebra.

Matmul operands are bf16 (halves DMA, full PE rate); PSUM accumulation is
fp32.  Inputs stream over three DMA queues ordered by consumption (wz
first on two queues, U-powers pipelined behind), and the output leaves in
natural [10,256] layout via a single 10-descriptor DMA.  Set
BASS_GNN_DT=float32r for fp32-precision operands, BASS_GNN_K to override
the truncation depth.
"""

import os

import numpy as np

N_NODES = 10
D = 256
N_CORES = 8
K_CAP = 50


def _pick_K(U):
    """Smallest K with ||U^{K+1}|| <= 8e-3 ||U|| (floor 2, cap K_CAP).

    Truncation error is ~||U^{K+1}||/||U|| relative; 8e-3 combined with the
    bf16 operand-rounding noise (~4e-3) keeps the end-to-end error under
    ~1e-2, a >2x margin on the 2e-2 accuracy gate.  K=3 for the generated
    inputs (measured 5.4e-3); each extra power costs ~130KB of U-power DMA
    on the critical input stream (~0.5us).
    """
    ko = os.environ.get("BASS_GNN_K")
    if ko:
        return int(ko)
    Uf = U.astype(np.float64)
    s1 = np.linalg.norm(Uf, 2)
    if s1 == 0.0:
        return 2
    P = Uf.copy()
    for k in range(1, K_CAP + 2):
        if np.linalg.norm(P, 2) <= 8e-3 * s1:
            return min(max(k - 1, 2), K_CAP)
        P = P @ Uf
    return None  # pathological; caller falls back to exact host scan


def _host_exact_scan(node_feat, edge_feat, edge_list, W1, W2, U):
    # Unreachable for the intended input distribution (spectral radius of
    # updateNN ~0.16); safety net for arbitrary U where no truncation exists.
    msg = (edge_feat @ W1) @ W2.T
    src, snk = edge_list[0], edge_list[1]
    deg = np.zeros(N_NODES, np.float32)
    np.add.at(deg, src, 1.0)
    np.add.at(deg, snk, 1.0)
    inv_deg = (1.0 / np.maximum(deg, 1.0)).astype(np.float32)
    state = node_feat.copy()
    for e in range(edge_feat.shape[0]):
        s, t = src[e], snk[e]
        me = msg[e]
        state[s] = (state[s] + inv_deg[s] * me * node_feat[t]) @ U
        state[t] = (state[t] + inv_deg[t] * me * node_feat[s]) @ U
    return state


def _apply_tile_patch():
    """Two workarounds for this walrus build / single-shot NEFF usage:

    1. Walrus here rejects >1 sync wait on ordinary instructions ("Too many
       sync wait commands"), but Tile's semaphore assignment attaches up to
       2.  Split the excess waits onto same-engine NOPs inserted immediately
       before the instruction (same stream, waits still execute before it).

    2. The kernel tail: keep the quiesce drain (with its waits — this is
       what guarantees the output DMA has landed) but skip the two
       all-engine barriers and the per-semaphore serial clear loop.  The
       clears only matter for re-executing the same NEFF; the NEFF-level
       epilogue observed on this toolchain resets all 256 semaphores anyway,
       so this is safe even under re-execution.  BASS_GNN_TRIM=0 restores
       them.
    """
    import concourse.mybir as mybir
    import concourse.tile as tile
    from bass_rust import ScopedClock

    if getattr(tile.TileContext, "_wait_split_patch", False):
        return

    orig_add = tile.TileContext._add_instruction

    def _split_add(self, inst):
        si = inst.sync_info
        if (
            si
            and si.on_wait
            and len(si.on_wait) > 1
            and not isinstance(inst, mybir.InstEventSemaphore)
        ):
            waits = list(si.on_wait)
            for w in waits[1:]:
                nop = mybir.InstNoOp(
                    name=self.nc.get_next_instruction_name(), ins=[], outs=[]
                )
                nop.engine = inst.engine
                nop.sync_info = mybir.SyncInfo(on_wait=[w], on_update=[])
                orig_add(self, nop)
            si.on_wait = waits[:1]
        orig_add(self, inst)

    trim = os.environ.get("BASS_GNN_TRIM", "1") != "0"

    def _patched_drain(self, tick_clock, wait_clock):
        nc = self.nc
        drain_inst = nc.sync.drain()
        wait_clock.add_sem_waits(
            drain_inst.ins, ScopedClock({None: tick_clock.global_clock})
        )
        si = drain_inst.ins.sync_info
        waits = list(si.on_wait) if si and si.on_wait else []
        if len(waits) > 1:
            si.on_wait = waits[:1]
            for w in waits[1:]:
                nop = nc.sync.nop()
                nop.ins.sync_info = mybir.SyncInfo(on_wait=[w], on_update=[])
        assert self.sems is not None
        popped = nc._tile_sem_poison_stack.pop()
        assert popped is self._sem_poison
        if trim:
            return
        nc.all_engine_barrier()
        nc.clear_and_free_semaphores(list(self.sems.allocated().values()))
        nc.all_engine_barrier()

    tile.TileContext._add_instruction = _split_add
    tile.TileContext._drain_and_barrier = _patched_drain
    tile.TileContext._wait_split_patch = True


def _ensure_axon_profile_hook():
    """This image's ``antenv`` package lacks ``axon_hooks``; bass_utils
    crashes on ``from antenv.axon_hooks import ...`` if tracing is requested
    (BASS_TRACE=1).  Install the module shim, wired to the ctypes NTFF hook
    from trn_agent_boot when available, so tracing works (or degrades
    gracefully instead of raising)."""
    import sys
    import types

    if "antenv.axon_hooks" in sys.modules:
        return
    mod = types.ModuleType("antenv.axon_hooks")
    mod._hook = None

    def set_axon_ntff_profile_hook(h):
        mod._hook = h

    def get_axon_ntff_profile_hook():
        return mod._hook

    mod.set_axon_ntff_profile_hook = set_axon_ntff_profile_hook
    mod.get_axon_ntff_profile_hook = get_axon_ntff_profile_hook
    try:
        import antenv

        antenv.axon_hooks = mod
    except ImportError:
        pass
    sys.modules["antenv.axon_hooks"] = mod
    try:
        from trn_agent_boot.trn_boot import _ntff_profile_via_ctypes

        mod._hook = _ntff_profile_via_ctypes("/opt/axon/libaxon_pjrt.so")
    except Exception:
        pass  # hook stays None; bass_utils logs and skips tracing


def _prune_init_barrier(nc):
    """Remove the all-engine barrier Bass.__init__ emits after the engine
    preambles.  It only orders cross-engine preamble effects (the const-pool
    memsets, which _prune_unused_const_memsets already dropped) before the
    body; each engine's own preamble stays ordered with its body in-stream.
    Saves the serial gather/release handshake (~0.7us) at body entry."""
    import concourse.mybir as mybir

    def is_barrier_sync(si):
        if si is None:
            return False
        for w in list(si.on_wait or []) + list(si.on_update or []):
            if str(getattr(w, "ant_name", "")).startswith("barrier_"):
                return True
        return False

    b = nc.main_func.blocks[0]
    insts = list(b.instructions)
    out = []
    prefix = True
    for i in insts:
        if prefix:
            if isinstance(i, (mybir.InstCall, mybir.InstRegisterMove)):
                out.append(i)
                continue
            if isinstance(i, mybir.InstMemset):
                return  # const memsets survived pruning: barrier still needed
            if isinstance(i, mybir.InstEventSemaphore) and is_barrier_sync(
                i.sync_info
            ):
                continue  # drop barrier sem op
            if isinstance(i, mybir.InstDrain) and (
                i.sync_info is None or is_barrier_sync(i.sync_info)
            ):
                continue  # drop barrier drain
            prefix = False  # first real body instruction
        out.append(i)
    if len(out) != len(insts):
        b.instructions = out


def _prune_unused_const_memsets(nc):
    """Drop the framework's const-pool MEMSETs (emitted unconditionally by
    Bass.__init__) when nothing in the program reads them.  They sit on the
    GpSimd stream before the body handshake; removing them starts every
    engine's body ~0.4us earlier."""
    import concourse.mybir as mybir

    def ap_tensor_names(args):
        names = set()
        for a in args:
            t = getattr(getattr(a, "bass_ap", None), "tensor", None)
            if t is not None:
                names.add(t.name)
            mr = getattr(a, "memref", None)
            if isinstance(mr, str):
                names.add(mr)
        return names

    read_names = set()
    for b in nc.main_func.blocks:
        for i in b.instructions:
            read_names |= ap_tensor_names(i.ins)
    for b in nc.main_func.blocks:
        insts = list(b.instructions)
        keep = [
            i
            for i in insts
            if not (
                isinstance(i, mybir.InstMemset)
                and i.sync_info is None
                and ap_tensor_names(i.outs)
                and all(
                    n.startswith("const-") and n not in read_names
                    for n in ap_tensor_names(i.outs)
                )
            )
        ]
        if len(keep) != len(insts):
            b.instructions = keep


def _build_program(K):
    import concourse.bass as bass
    import concourse.mybir as mybir
    import concourse.tile as tile

    _apply_tile_patch()

    S = K * N_NODES
    f32 = mybir.dt.float32
    mdt = getattr(mybir.dt, os.environ.get("BASS_GNN_DT", "bfloat16"))

    # The NRT pseudo sync barrier expands (at NEFF load) into a ~2.9us
    # runtime-event wait before any engine runs.  Under the axon/PJRT
    # execution path the runtime stages all input DRAM buffers before the
    # NEFF starts, so the barrier only adds dead time.  BASS_GNN_NRTB=1
    # restores it.
    skip_nrtb = os.environ.get("BASS_GNN_NRTB", "0") != "1"
    n_run = int(os.environ.get("BASS_GNN_CORES", "1"))
    orig_nrtb = bass.Bass._nrt_pseudo_barrier
    if skip_nrtb:
        bass.Bass._nrt_pseudo_barrier = lambda self: None
    try:
        nc = bass.Bass(
            "TRN2", debug=False, num_devices=n_run, enable_partition_id=False
        )
    finally:
        bass.Bass._nrt_pseudo_barrier = orig_nrtb
    # wz columns: [EselT h0|h1 (S each) | WcT h0|h1 (256 each) | nfs h0|h1]
    # — everything the msgT+b stage needs.
    CW = 2 * S + 2 * D + 2 * S
    wz_d = nc.dram_tensor("wz", [128, CW], mdt, kind="ExternalInput")
    # U powers in consumption order: block b = 2*j + a holds U^{j+1} rows
    # [128a:128(a+1)].  Each block is its own DMA so the accumulation chain
    # only waits for the block it is about to consume.
    NB = 2 * K
    ubz_d = nc.dram_tensor("ubz", [128, NB * D], mdt, kind="ExternalInput")
    basez_d = nc.dram_tensor("basez", [N_NODES, D], f32, kind="ExternalInput")
    outz_d = nc.dram_tensor("outz", [N_NODES, D], f32, kind="ExternalOutput")

    with tile.TileContext(nc) as tc:
        with (
            tc.tile_pool(name="singles", bufs=1) as sg,
            tc.tile_pool(name="mm_psum", bufs=2, space=bass.MemorySpace.PSUM) as mmp,
            tc.tile_pool(name="o_psum", bufs=1, space=bass.MemorySpace.PSUM) as opp,
        ):
            wz = sg.tile([128, CW], mdt)
            ubz = sg.tile([128, NB * D], mdt)
            bz = sg.tile([N_NODES, D], f32)
            # Streaming choreography.  Aggregate input (~565KB) is HBM-bound
            # (~55GB/s per queue with all three active), so arrival order
            # must match consumption order: a wz slice rides FIRST on every
            # queue (msgT gates everything), then the U-power blocks
            # round-robin across the queues in chain order.
            engs = [nc.sync, nc.scalar, nc.gpsimd]
            # slice cuts aligned to consumer boundaries: slice1 covers msgT
            # chunk-1's operands (esel0 + wct0 cols 0:128), slice2 the
            # remaining wct columns for chunks 2-3, slice3 chunk-4 + nfs.
            cuts = [0, 2 * S + 128, 2 * S + D + 128, CW]
            for q in range(3):
                engs[q].dma_start(
                    wz[:, cuts[q] : cuts[q + 1]], wz_d[:, cuts[q] : cuts[q + 1]]
                )
            for b in range(NB):
                sl = slice(b * D, (b + 1) * D)
                engs[b % 3].dma_start(ubz[:, sl], ubz_d[:, sl])
            nc.gpsimd.dma_start(bz[:], basez_d[:])

            eselt = [wz[:, kc * S : (kc + 1) * S] for kc in range(2)]
            wct = [wz[:, 2 * S + kc * D : 2 * S + (kc + 1) * D] for kc in range(2)]
            o_nfs = 2 * S + 2 * D
            nfs = [wz[:, o_nfs + a * S : o_nfs + (a + 1) * S] for a in range(2)]

            # msgT[half a] = (Wc @ EselT)[128a:128(a+1), :]   (Wc = W2 W1^T)
            bt = sg.tile([128, 2, S], mdt)
            pm = []
            for a in range(2):
                p = mmp.tile([128, S], f32, tag="pm")
                nc.tensor.matmul(
                    p[:], wct[0][:, 128 * a : 128 * (a + 1)], eselt[0],
                    start=True, stop=False,
                )
                nc.tensor.matmul(
                    p[:], wct[1][:, 128 * a : 128 * (a + 1)], eselt[1],
                    start=False, stop=True,
                )
                pm.append(p)
            # bT = msgT * nfsT (DVE, reads msgT from PSUM).  Emitted per
            # (j, half) in chain order: the first accumulation matmul only
            # needs bt columns 0:10, so producing those first lets the PE
            # chain start one DVE-op earlier.
            for j in range(K):
                for a in range(2):
                    sl = slice(j * N_NODES, (j + 1) * N_NODES)
                    nc.vector.tensor_mul(bt[:, a, sl], pm[a][:, sl], nfs[a][:, sl])

            # out[10,256] = sum_{a,j} bt[:,a,js].T @ U^{j+1}[half a]
            # j-major emission matches the ubz block arrival order.
            ph = opp.tile([N_NODES, D], f32, tag="oz")
            for b in range(NB):
                j, a = b // 2, b % 2
                nc.tensor.matmul(
                    ph[:],
                    bt[:, a, j * N_NODES : (j + 1) * N_NODES],
                    ubz[:, b * D : (b + 1) * D],
                    start=(b == 0),
                    stop=(b == NB - 1),
                )

            # Final base add split across two engines: Scalar handles the
            # second half right before it issues the output DMA (program
            # order covers that dependency, so the DMA only needs one
            # cross-engine wait on Vector's half).
            oz = sg.tile([N_NODES, D], f32)
            nc.vector.tensor_add(oz[:], ph[:], bz[:])
            # GpSimd's software-DGE trigger measures ~650ns vs ~950ns for
            # the hardware-DGE output trigger on Sync/Scalar, and GpSimd is
            # idle once its input triggers retire.
            nc.gpsimd.dma_start(outz_d[:], oz[:])

    _prune_unused_const_memsets(nc)
    # A/B-measured: removing the init all-engine barrier starts the body
    # ~0.6us earlier but consistently costs ~1.2us downstream (walrus
    # reschedules the NEFF-level epilogue sync into the body).  Keep it.
    if os.environ.get("BASS_GNN_AEB", "1") != "1":
        _prune_init_barrier(nc)
    nc.finalize()
    return nc


def kernel(node_feat, edge_feat, edge_list, intsc_feat_fc, messageNN, updateNN):
    node_feat = np.ascontiguousarray(np.asarray(node_feat, np.float32))
    edge_feat = np.ascontiguousarray(np.asarray(edge_feat, np.float32))
    edge_list = np.asarray(edge_list)
    W1 = np.ascontiguousarray(np.asarray(intsc_feat_fc, np.float32))
    W2 = np.ascontiguousarray(np.asarray(messageNN, np.float32))
    U = np.ascontiguousarray(np.asarray(updateNN, np.float32))
    E = edge_feat.shape[0]

    K = _pick_K(U)
    if K is None:
        return _host_exact_scan(node_feat, edge_feat, edge_list, W1, W2, U)
    S = K * N_NODES

    # ---- host index preprocessing (integer bookkeeping + layout) ----
    src = edge_list[0].astype(np.int64)
    snk = edge_list[1].astype(np.int64)
    deg = (
        np.bincount(src, minlength=N_NODES) + np.bincount(snk, minlength=N_NODES)
    ).astype(np.float32)
    inv_deg = (1.0 / np.maximum(deg, 1.0)).astype(np.float32)
    m = deg.astype(np.int64)

    # touch stream: edge e -> touch 2e (node=src, partner=snk),
    #               touch 2e+1 (node=snk, partner=src)
    tnode = np.empty(2 * E, np.int64)
    tpart = np.empty(2 * E, np.int64)
    tedge = np.empty(2 * E, np.int64)
    tnode[0::2] = src
    tnode[1::2] = snk
    tpart[0::2] = snk
    tpart[1::2] = src
    tedge[0::2] = np.arange(E)
    tedge[1::2] = np.arange(E)

    order = np.argsort(tnode, kind="stable")
    starts = np.searchsorted(tnode[order], np.arange(N_NODES))
    k_idx = np.empty(2 * E, np.int64)
    k_idx[order] = np.arange(2 * E) - starts[tnode[order]] + 1
    jp = m[tnode] - k_idx  # j' index; keep the last K touches per node

    keep = jp < K
    kn, kp, ke, kj = tnode[keep], tpart[keep], tedge[keep], jp[keep]
    slot = kj * N_NODES + kn

    sel_edge = np.zeros(S, np.int64)
    sel_edge[slot] = ke
    EselT = np.ascontiguousarray(edge_feat[sel_edge].T)  # [D, S]
    nfsT = np.zeros((D, S), np.float32)  # [D, S]: col = inv_deg[n]*nf[partner]
    nfsT[:, slot] = (node_feat[kp] * inv_deg[kn][:, None]).T

    # weight-only folds (O(D^3), data-independent)
    WcT = np.ascontiguousarray((W1 @ W2.T).astype(np.float32))  # = (W2 W1^T)^T
    Uf = U.astype(np.float64)
    upows = []
    P = Uf.copy()
    for _ in range(K):
        upows.append(P.astype(np.float32))
        P = P @ Uf

    # low-degree boundary constant: nodes with m<=K keep a node_feat @ U^m
    # term (their initial state survives truncation); untouched nodes keep
    # node_feat itself.  Zero for every node when all degrees exceed K.
    base = np.zeros((N_NODES, D), np.float32)
    for n in range(N_NODES):
        if m[n] == 0:
            base[n] = node_feat[n]
        elif m[n] <= K:
            base[n] = node_feat[n] @ upows[m[n] - 1]

    # ---- device execution ----
    _ensure_axon_profile_hook()
    from concourse.bass_utils import run_bass_kernel_spmd

    np_mdt = {"bfloat16": "bfloat16", "float32r": np.float32, "float32": np.float32}[
        os.environ.get("BASS_GNN_DT", "bfloat16")
    ]
    if np_mdt == "bfloat16":
        import ml_dtypes

        np_mdt = ml_dtypes.bfloat16

    nc = _build_program(K)
    wz = np.concatenate(
        [EselT[0:128], EselT[128:256], WcT[0:128], WcT[128:256],
         nfsT[0:128], nfsT[128:256]],
        axis=1,
    ).astype(np_mdt)
    # ubz blocks in chain-consumption order: block 2*j+a = U^{j+1} half a
    ubz = np.concatenate(
        [upows[j][128 * a : 128 * (a + 1), :] for j in range(K) for a in range(2)],
        axis=1,
    ).astype(np_mdt)
    in_map = {
        "wz": np.ascontiguousarray(wz),
        "ubz": np.ascontiguousarray(ubz),
        "basez": base,
    }
    # Replicated SPMD on all 8 cores; output read from core 0.  Measured:
    # running core 0 alone is ~2us SLOWER despite less HBM contention — the
    # idle cores drop the chip into a lower clock state.
    n_run = int(os.environ.get("BASS_GNN_CORES", "8"))
    in_maps = [dict(in_map) for _ in range(n_run)]
    res = run_bass_kernel_spmd(nc, in_maps, list(range(n_run)))
    out = np.ascontiguousarray(res.results[0]["outz"]).astype(np.float32, copy=False)
    kernel.last_results = res
    return out


# revision 38
# speedup vs baseline: 1.0080x; 1.0080x over previous
"""Trainium2 Bass kernel for nn_Evo_Path_GNN (gnn_message_passing).

Algorithm
---------
The reference runs a 50000-step sequential scan over edges on a [10, 256]
state.  Each step is affine in the state row it touches:

    state[n] <- (state[n] + b) @ U        (one "touch"; 2 touches per edge)

with b = inv_deg[n] * msg[e] * node_feat[partner].  Unrolling per node, the
final row is

    out[n] = node_feat[n] @ U^{m_n} + sum_k b_{n,k} @ U^{m_n - k + 1}

where m_n is the number of touches of node n and k the touch order.  U is
0.01-scaled gaussian (||U^k|| decays ~5x per power), so terms older than a
few touches are below the accuracy gate.  We keep only the last K touches
per node (K chosen at runtime from the measured norms of U^k), which
converts the 100k-long serial chain into the closed form

    out[n] = sum_{j'=0}^{K-1} P_{n,j'} @ U^{j'+1} + base_n

with P_{n,j'} the b-vector of the (m_n - j')-th touch of node n.  Unlike the
earlier Horner-recursion kernel, this evaluates the sum DIRECTLY as a flat
accumulation of K*2 independent PSUM-accumulating matmuls

    out[10,256] += bt[:, a, j-slice].T @ Upow[j][half a]    (a=0,1; j=0..K-1)

with the tiny [128,10] bt slices as the PE *stationary* operand and the
U-powers as 256-wide *moving* operands.  This removes the serial PE
dependency chain, produces the output in natural [10, 256] layout (10 fat
DMA descriptors instead of 256 40-byte ones), and needs no Horner adds.
U powers and Wc = messageNN @ intsc_feat_fc^T are folded on the host
(weight-only O(D^3) preprocessing); the host also does all integer
bookkeeping (touch ordering, slot permutation, gathers) and the low-degree
boundary constants (node_feat @ U^m for nodes with m <= K touches).  The
device does all per-edge feature math of the truncated algorithm: the
message projection matmuls, the b-vector products, and the K-power update
algebra.

Matmul operands are bf16 (halves DMA, full PE rate); PSUM accumulation is
fp32.  Inputs stream over three DMA queues ordered by consumption (wz
first on two queues, U-powers pipelined behind), and the output leaves in
natural [10,256] layout via a single 10-descriptor DMA.  Set
BASS_GNN_DT=float32r for fp32-precision operands, BASS_GNN_K to override
the truncation depth.
"""

import os

import numpy as np

N_NODES = 10
D = 256
N_CORES = 8
K_CAP = 50


def _pick_K(U):
    """Smallest K with ||U^{K+1}|| <= 8e-3 ||U|| (floor 2, cap K_CAP).

    Truncation error is ~||U^{K+1}||/||U|| relative; 8e-3 combined with the
    bf16 operand-rounding noise (~4e-3) keeps the end-to-end error under
    ~1e-2, a >2x margin on the 2e-2 accuracy gate.  K=3 for the generated
    inputs (measured 5.4e-3); each extra power costs ~130KB of U-power DMA
    on the critical input stream (~0.5us).
    """
    ko = os.environ.get("BASS_GNN_K")
    if ko:
        return int(ko)
    Uf = U.astype(np.float64)
    s1 = np.linalg.norm(Uf, 2)
    if s1 == 0.0:
        return 2
    P = Uf.copy()
    for k in range(1, K_CAP + 2):
        if np.linalg.norm(P, 2) <= 8e-3 * s1:
            return min(max(k - 1, 2), K_CAP)
        P = P @ Uf
    return None  # pathological; caller falls back to exact host scan


def _host_exact_scan(node_feat, edge_feat, edge_list, W1, W2, U):
    # Unreachable for the intended input distribution (spectral radius of
    # updateNN ~0.16); safety net for arbitrary U where no truncation exists.
    msg = (edge_feat @ W1) @ W2.T
    src, snk = edge_list[0], edge_list[1]
    deg = np.zeros(N_NODES, np.float32)
    np.add.at(deg, src, 1.0)
    np.add.at(deg, snk, 1.0)
    inv_deg = (1.0 / np.maximum(deg, 1.0)).astype(np.float32)
    state = node_feat.copy()
    for e in range(edge_feat.shape[0]):
        s, t = src[e], snk[e]
        me = msg[e]
        state[s] = (state[s] + inv_deg[s] * me * node_feat[t]) @ U
        state[t] = (state[t] + inv_deg[t] * me * node_feat[s]) @ U
    return state


def _apply_tile_patch():
    """Two workarounds for this walrus build / single-shot NEFF usage:

    1. Walrus here rejects >1 sync wait on ordinary instructions ("Too many
       sync wait commands"), but Tile's semaphore assignment attaches up to
       2.  Split the excess waits onto same-engine NOPs inserted immediately
       before the instruction (same stream, waits still execute before it).

    2. The kernel tail: keep the quiesce drain (with its waits — this is
       what guarantees the output DMA has landed) but skip the two
       all-engine barriers and the per-semaphore serial clear loop.  The
       clears only matter for re-executing the same NEFF; the NEFF-level
       epilogue observed on this toolchain resets all 256 semaphores anyway,
       so this is safe even under re-execution.  BASS_GNN_TRIM=0 restores
       them.
    """
    import concourse.mybir as mybir
    import concourse.tile as tile
    from bass_rust import ScopedClock

    if getattr(tile.TileContext, "_wait_split_patch", False):
        return

    orig_add = tile.TileContext._add_instruction

    def _split_add(self, inst):
        si = inst.sync_info
        if (
            si
            and si.on_wait
            and len(si.on_wait) > 1
            and not isinstance(inst, mybir.InstEventSemaphore)
        ):
            waits = list(si.on_wait)
            for w in waits[1:]:
                nop = mybir.InstNoOp(
                    name=self.nc.get_next_instruction_name(), ins=[], outs=[]
                )
                nop.engine = inst.engine
                nop.sync_info = mybir.SyncInfo(on_wait=[w], on_update=[])
                orig_add(self, nop)
            si.on_wait = waits[:1]
        orig_add(self, inst)

    trim = os.environ.get("BASS_GNN_TRIM", "1") != "0"

    def _patched_drain(self, tick_clock, wait_clock):
        nc = self.nc
        drain_inst = nc.sync.drain()
        wait_clock.add_sem_waits(
            drain_inst.ins, ScopedClock({None: tick_clock.global_clock})
        )
        si = drain_inst.ins.sync_info
        waits = list(si.on_wait) if si and si.on_wait else []
        if len(waits) > 1:
            si.on_wait = waits[:1]
            for w in waits[1:]:
                nop = nc.sync.nop()
                nop.ins.sync_info = mybir.SyncInfo(on_wait=[w], on_update=[])
        assert self.sems is not None
        popped = nc._tile_sem_poison_stack.pop()
        assert popped is self._sem_poison
        if trim:
            return
        nc.all_engine_barrier()
        nc.clear_and_free_semaphores(list(self.sems.allocated().values()))
        nc.all_engine_barrier()

    tile.TileContext._add_instruction = _split_add
    tile.TileContext._drain_and_barrier = _patched_drain
    tile.TileContext._wait_split_patch = True


def _ensure_axon_profile_hook():
    """This image's ``antenv`` package lacks ``axon_hooks``; bass_utils
    crashes on ``from antenv.axon_hooks import ...`` if tracing is requested
    (BASS_TRACE=1).  Install the module shim, wired to the ctypes NTFF hook
    from trn_agent_boot when available, so tracing works (or degrades
    gracefully instead of raising)."""
    import sys
    import types

    if "antenv.axon_hooks" in sys.modules:
        return
    mod = types.ModuleType("antenv.axon_hooks")
    mod._hook = None

    def set_axon_ntff_profile_hook(h):
        mod._hook = h

    def get_axon_ntff_profile_hook():
        return mod._hook

    mod.set_axon_ntff_profile_hook = set_axon_ntff_profile_hook
    mod.get_axon_ntff_profile_hook = get_axon_ntff_profile_hook
    try:
        import antenv

        antenv.axon_hooks = mod
    except ImportError:
        pass
    sys.modules["antenv.axon_hooks"] = mod
    try:
        from trn_agent_boot.trn_boot import _ntff_profile_via_ctypes

        mod._hook = _ntff_profile_via_ctypes("/opt/axon/libaxon_pjrt.so")
    except Exception:
        pass  # hook stays None; bass_utils logs and skips tracing


def _prune_init_barrier(nc):
    """Remove the all-engine barrier Bass.__init__ emits after the engine
    preambles.  It only orders cross-engine preamble effects (the const-pool
    memsets, which _prune_unused_const_memsets already dropped) before the
    body; each engine's own preamble stays ordered with its body in-stream.
    Saves the serial gather/release handshake (~0.7us) at body entry."""
    import concourse.mybir as mybir

    def is_barrier_sync(si):
        if si is None:
            return False
        for w in list(si.on_wait or []) + list(si.on_update or []):
            if str(getattr(w, "ant_name", "")).startswith("barrier_"):
                return True
        return False

    b = nc.main_func.blocks[0]
    insts = list(b.instructions)
    out = []
    prefix = True
    for i in insts:
        if prefix:
            if isinstance(i, (mybir.InstCall, mybir.InstRegisterMove)):
                out.append(i)
                continue
            if isinstance(i, mybir.InstMemset):
                return  # const memsets survived pruning: barrier still needed
            if isinstance(i, mybir.InstEventSemaphore) and is_barrier_sync(
                i.sync_info
            ):
                continue  # drop barrier sem op
            if isinstance(i, mybir.InstDrain) and (
                i.sync_info is None or is_barrier_sync(i.sync_info)
            ):
                continue  # drop barrier drain
            prefix = False  # first real body instruction
        out.append(i)
    if len(out) != len(insts):
        b.instructions = out


def _prune_unused_const_memsets(nc):
    """Drop the framework's const-pool MEMSETs (emitted unconditionally by
    Bass.__init__) when nothing in the program reads them.  They sit on the
    GpSimd stream before the body handshake; removing them starts every
    engine's body ~0.4us earlier."""
    import concourse.mybir as mybir

    def ap_tensor_names(args):
        names = set()
        for a in args:
            t = getattr(getattr(a, "bass_ap", None), "tensor", None)
            if t is not None:
                names.add(t.name)
            mr = getattr(a, "memref", None)
            if isinstance(mr, str):
                names.add(mr)
        return names

    read_names = set()
    for b in nc.main_func.blocks:
        for i in b.instructions:
            read_names |= ap_tensor_names(i.ins)
    for b in nc.main_func.blocks:
        insts = list(b.instructions)
        keep = [
            i
            for i in insts
            if not (
                isinstance(i, mybir.InstMemset)
                and i.sync_info is None
                and ap_tensor_names(i.outs)
                and all(
                    n.startswith("const-") and n not in read_names
                    for n in ap_tensor_names(i.outs)
                )
            )
        ]
        if len(keep) != len(insts):
            b.instructions = keep


def _build_program(K):
    import concourse.bass as bass
    import concourse.mybir as mybir
    import concourse.tile as tile

    _apply_tile_patch()

    S = K * N_NODES
    f32 = mybir.dt.float32
    mdt = getattr(mybir.dt, os.environ.get("BASS_GNN_DT", "bfloat16"))

    # The NRT pseudo sync barrier expands (at NEFF load) into a ~2.9us
    # runtime-event wait before any engine runs.  Under the axon/PJRT
    # execution path the runtime stages all input DRAM buffers before the
    # NEFF starts, so the barrier only adds dead time.  BASS_GNN_NRTB=1
    # restores it.
    skip_nrtb = os.environ.get("BASS_GNN_NRTB", "0") != "1"
    n_run = int(os.environ.get("BASS_GNN_CORES", "1"))
    orig_nrtb = bass.Bass._nrt_pseudo_barrier
    if skip_nrtb:
        bass.Bass._nrt_pseudo_barrier = lambda self: None
    try:
        nc = bass.Bass(
            "TRN2", debug=False, num_devices=n_run, enable_partition_id=False
        )
    finally:
        bass.Bass._nrt_pseudo_barrier = orig_nrtb
    # wz columns: [EselT h0|h1 (S each) | WcT h0|h1 (256 each) | nfs h0|h1]
    # — everything the msgT+b stage needs.
    CW = 2 * S + 2 * D + 2 * S
    wz_d = nc.dram_tensor("wz", [128, CW], mdt, kind="ExternalInput")
    # U powers in consumption order: block b = 2*j + a holds U^{j+1} rows
    # [128a:128(a+1)].  Each block is its own DMA so the accumulation chain
    # only waits for the block it is about to consume.
    NB = 2 * K
    ubz_d = nc.dram_tensor("ubz", [128, NB * D], mdt, kind="ExternalInput")
    basez_d = nc.dram_tensor("basez", [N_NODES, D], f32, kind="ExternalInput")
    outz_d = nc.dram_tensor("outz", [N_NODES, D], f32, kind="ExternalOutput")

    with tile.TileContext(nc) as tc:
        with (
            tc.tile_pool(name="singles", bufs=1) as sg,
            tc.tile_pool(name="mm_psum", bufs=2, space=bass.MemorySpace.PSUM) as mmp,
            tc.tile_pool(name="o_psum", bufs=1, space=bass.MemorySpace.PSUM) as opp,
        ):
            wz = sg.tile([128, CW], mdt)
            ubz = sg.tile([128, NB * D], mdt)
            bz = sg.tile([N_NODES, D], f32)
            # Streaming choreography.  Aggregate input (~565KB) is HBM-bound
            # (~55GB/s per queue with all three active), so arrival order
            # must match consumption order: a wz slice rides FIRST on every
            # queue (msgT gates everything), then the U-power blocks
            # round-robin across the queues in chain order.
            engs = [nc.sync, nc.scalar, nc.gpsimd]
            # wz column order puts the a0-path operands first (they gate the
            # accumulation-chain start): slice1 = esel0|wct_a0k0|esel1,
            # slice2 = wct_a0k1|nfs0 (completes pm-a0 AND the first bt mul),
            # slice3 = the a1 weight blocks + nfs1.
            cuts = [0, 2 * S + 128, 3 * S + 2 * 128, CW]
            for q in range(3):
                engs[q].dma_start(
                    wz[:, cuts[q] : cuts[q + 1]], wz_d[:, cuts[q] : cuts[q + 1]]
                )
            for b in range(NB):
                sl = slice(b * D, (b + 1) * D)
                engs[b % 3].dma_start(ubz[:, sl], ubz_d[:, sl])
            nc.gpsimd.dma_start(bz[:], basez_d[:])

            # column map (see host-side assembly):
            # [esel0 S | wct_a0k0 128 | esel1 S | wct_a0k1 128 | nfs0 S |
            #  wct_a1k0 128 | wct_a1k1 128 | nfs1 S]
            eselt = [wz[:, 0:S], wz[:, S + 128 : 2 * S + 128]]
            o2 = 2 * S + 128
            o3 = 3 * S + 2 * 128
            wblk = {
                (0, 0): wz[:, S : S + 128],
                (0, 1): wz[:, o2 : o2 + 128],
                (1, 0): wz[:, o3 : o3 + 128],
                (1, 1): wz[:, o3 + 128 : o3 + 2 * 128],
            }
            nfs = [wz[:, o2 + 128 : o2 + 128 + S], wz[:, o3 + 2 * 128 : CW]]

            # msgT[half a] = (Wc @ EselT)[128a:128(a+1), :]   (Wc = W2 W1^T)
            bt = sg.tile([128, 2, S], mdt)
            pm = []
            for a in range(2):
                p = mmp.tile([128, S], f32, tag="pm")
                nc.tensor.matmul(
                    p[:], wblk[(a, 0)], eselt[0], start=True, stop=False
                )
                nc.tensor.matmul(
                    p[:], wblk[(a, 1)], eselt[1], start=False, stop=True
                )
                pm.append(p)
            # bT = msgT * nfsT (DVE, reads msgT from PSUM).  Emitted per
            # (j, half) in chain order: the first accumulation matmul only
            # needs bt columns 0:10, so producing those first lets the PE
            # chain start one DVE-op earlier.
            for j in range(K):
                for a in range(2):
                    sl = slice(j * N_NODES, (j + 1) * N_NODES)
                    nc.vector.tensor_mul(bt[:, a, sl], pm[a][:, sl], nfs[a][:, sl])

            # out[10,256] = sum_{a,j} bt[:,a,js].T @ U^{j+1}[half a]
            # j-major emission matches the ubz block arrival order.
            ph = opp.tile([N_NODES, D], f32, tag="oz")
            for b in range(NB):
                j, a = b // 2, b % 2
                nc.tensor.matmul(
                    ph[:],
                    bt[:, a, j * N_NODES : (j + 1) * N_NODES],
                    ubz[:, b * D : (b + 1) * D],
                    start=(b == 0),
                    stop=(b == NB - 1),
                )

            # Final base add split across two engines: Scalar handles the
            # second half right before it issues the output DMA (program
            # order covers that dependency, so the DMA only needs one
            # cross-engine wait on Vector's half).
            oz = sg.tile([N_NODES, D], f32)
            nc.vector.tensor_add(oz[:], ph[:], bz[:])
            # Output trigger stays on Scalar (hardware DGE): GpSimd's
            # software-DGE trigger executes faster (~650ns vs ~950ns) but
            # pays a ~400ns post-wait dispatch gap that cancels the gain.
            nc.scalar.dma_start(outz_d[:], oz[:])

    _prune_unused_const_memsets(nc)
    # A/B-measured: removing the init all-engine barrier starts the body
    # ~0.6us earlier but consistently costs ~1.2us downstream (walrus
    # reschedules the NEFF-level epilogue sync into the body).  Keep it.
    if os.environ.get("BASS_GNN_AEB", "1") != "1":
        _prune_init_barrier(nc)
    nc.finalize()
    return nc


def kernel(node_feat, edge_feat, edge_list, intsc_feat_fc, messageNN, updateNN):
    node_feat = np.ascontiguousarray(np.asarray(node_feat, np.float32))
    edge_feat = np.ascontiguousarray(np.asarray(edge_feat, np.float32))
    edge_list = np.asarray(edge_list)
    W1 = np.ascontiguousarray(np.asarray(intsc_feat_fc, np.float32))
    W2 = np.ascontiguousarray(np.asarray(messageNN, np.float32))
    U = np.ascontiguousarray(np.asarray(updateNN, np.float32))
    E = edge_feat.shape[0]

    K = _pick_K(U)
    if K is None:
        return _host_exact_scan(node_feat, edge_feat, edge_list, W1, W2, U)
    S = K * N_NODES

    # ---- host index preprocessing (integer bookkeeping + layout) ----
    src = edge_list[0].astype(np.int64)
    snk = edge_list[1].astype(np.int64)
    deg = (
        np.bincount(src, minlength=N_NODES) + np.bincount(snk, minlength=N_NODES)
    ).astype(np.float32)
    inv_deg = (1.0 / np.maximum(deg, 1.0)).astype(np.float32)
    m = deg.astype(np.int64)

    # touch stream: edge e -> touch 2e (node=src, partner=snk),
    #               touch 2e+1 (node=snk, partner=src)
    tnode = np.empty(2 * E, np.int64)
    tpart = np.empty(2 * E, np.int64)
    tedge = np.empty(2 * E, np.int64)
    tnode[0::2] = src
    tnode[1::2] = snk
    tpart[0::2] = snk
    tpart[1::2] = src
    tedge[0::2] = np.arange(E)
    tedge[1::2] = np.arange(E)

    order = np.argsort(tnode, kind="stable")
    starts = np.searchsorted(tnode[order], np.arange(N_NODES))
    k_idx = np.empty(2 * E, np.int64)
    k_idx[order] = np.arange(2 * E) - starts[tnode[order]] + 1
    jp = m[tnode] - k_idx  # j' index; keep the last K touches per node

    keep = jp < K
    kn, kp, ke, kj = tnode[keep], tpart[keep], tedge[keep], jp[keep]
    slot = kj * N_NODES + kn

    sel_edge = np.zeros(S, np.int64)
    sel_edge[slot] = ke
    EselT = np.ascontiguousarray(edge_feat[sel_edge].T)  # [D, S]
    nfsT = np.zeros((D, S), np.float32)  # [D, S]: col = inv_deg[n]*nf[partner]
    nfsT[:, slot] = (node_feat[kp] * inv_deg[kn][:, None]).T

    # weight-only folds (O(D^3), data-independent)
    WcT = np.ascontiguousarray((W1 @ W2.T).astype(np.float32))  # = (W2 W1^T)^T
    Uf = U.astype(np.float64)
    upows = []
    P = Uf.copy()
    for _ in range(K):
        upows.append(P.astype(np.float32))
        P = P @ Uf

    # low-degree boundary constant: nodes with m<=K keep a node_feat @ U^m
    # term (their initial state survives truncation); untouched nodes keep
    # node_feat itself.  Zero for every node when all degrees exceed K.
    base = np.zeros((N_NODES, D), np.float32)
    for n in range(N_NODES):
        if m[n] == 0:
            base[n] = node_feat[n]
        elif m[n] <= K:
            base[n] = node_feat[n] @ upows[m[n] - 1]

    # ---- device execution ----
    _ensure_axon_profile_hook()
    from concourse.bass_utils import run_bass_kernel_spmd

    np_mdt = {"bfloat16": "bfloat16", "float32r": np.float32, "float32": np.float32}[
        os.environ.get("BASS_GNN_DT", "bfloat16")
    ]
    if np_mdt == "bfloat16":
        import ml_dtypes

        np_mdt = ml_dtypes.bfloat16

    nc = _build_program(K)
    # column order matches the device view map: a0-path operands first
    wz = np.concatenate(
        [EselT[0:128], WcT[0:128, 0:128], EselT[128:256], WcT[128:256, 0:128],
         nfsT[0:128], WcT[0:128, 128:256], WcT[128:256, 128:256],
         nfsT[128:256]],
        axis=1,
    ).astype(np_mdt)
    # ubz blocks in chain-consumption order: block 2*j+a = U^{j+1} half a
    ubz = np.concatenate(
        [upows[j][128 * a : 128 * (a + 1), :] for j in range(K) for a in range(2)],
        axis=1,
    ).astype(np_mdt)
    in_map = {
        "wz": np.ascontiguousarray(wz),
        "ubz": np.ascontiguousarray(ubz),
        "basez": base,
    }
    # Replicated SPMD on all 8 cores; output read from core 0.  Measured:
    # running core 0 alone is ~2us SLOWER despite less HBM contention — the
    # idle cores drop the chip into a lower clock state.
    n_run = int(os.environ.get("BASS_GNN_CORES", "8"))
    in_maps = [dict(in_map) for _ in range(n_run)]
    res = run_bass_kernel_spmd(nc, in_maps, list(range(n_run)))
    out = np.ascontiguousarray(res.results[0]["outz"]).astype(np.float32, copy=False)
    kernel.last_results = res
    return out
